# revision 40
# baseline (speedup 1.0000x reference)
"""Multi-head attention (B=4, S=2048, D=768, H=12) on 8 TRN2 NeuronCores.

Sharding: core i handles batch b = i//2 and head-group g = i%2 (6 heads of 64).
Each core computes Q/K/V projections for its head slice, attention, and a
partial output projection (row-slice of Wo). Host sums the two partials per
batch and adds bo.

Device layout (v2):
  - x fed pre-transposed as xT [D, S]; projections contract over D on the
    partition dim. Q, K produced transposed: QT/KT [384, S].
  - logitsT [k, q]: lhsT = KT_h [64, k-tile], rhs = QT_h [64, q-tile].
  - Mask folded into V instead of an exp bias: V rows and the softmax-ones
    column are multiplied by (1-mask), so masked keys drop out of both the
    PV numerator and the denominator. exp is then bias-free.
  - exp split across engines: ACT computes exact Exp on most k-chunks; the
    Pool (gpsimd) engine computes a Schraudolph-style approximate exp
    (bf16 bits = round(23.083*logit + 16249.25), via an int16 tensor_scalar)
    on ENG_POOL_KC chunks. This keeps exp off the PE critical path.
  - V kept in natural [k, 6, 65] layout (65th col = (1-mask)); the PV matmul
    accumulates ctxT [64, q] + denominator (row 64) per head in one group.
  - Normalization: reciprocal of denominator, DMA-broadcast across 64
    partitions, fused into the PSUM->SBUF extraction multiply.
  - Output projection contracts over HEAD PAIRS (contraction 128): ctx of
    the odd head is partition-shifted into rows 64..127 of a pair tile via
    an SBUF->SBUF DMA, halving Wo matmul count vs per-head contraction 64.
  - Wo matmuls for q-chunk qc-1 are spread as fillers through qc's kc loop
    (one per kc slot) so the PE never idles waiting on exp.
  - All matmul operands bf16 (fp32 PSUM accumulate).
"""

import numpy as np
from contextlib import ExitStack

S = 2048
D = 768
HL = 6  # heads per core
HD = 64
CPB = 384  # channels per core = HL * HD
DC = D // 128  # 6 contraction chunks
CC = CPB // 128  # 3 chunks of QT/KT partitions
NQ4 = S // 512  # 4 q chunks of 512
NK = S // 128  # 16 k chunks of 128
NEG_BIG = -1.0e9

# Schraudolph approx-exp constants (bf16 bit trick), softmax scale folded in.
SCH_SCALE = float(0.125 * 128 * np.log2(np.e))  # 23.0831...
SCH_BIAS = 16249.25

# exp split: ACT computes head 0 exactly; head 1 uses the Schraudolph
# approximation on Pool (even kc) / DVE (odd kc).

_cache = {}


def _build_nc(reps=1, parts="all"):
    import concourse.bass as bass
    import concourse.mybir as mybir
    import concourse.tile as tile
    from concourse import bacc
    from contextlib import nullcontext

    f32 = mybir.dt.float32
    bf16 = mybir.dt.bfloat16
    i16 = mybir.dt.int16
    AF = mybir.ActivationFunctionType

    nc = bacc.Bacc("TRN2", target_bir_lowering=False, debug=False,
                   enable_asserts=False)

    xt = nc.dram_tensor("xt", [D, S], bf16, kind="ExternalInput").ap()
    wq = nc.dram_tensor("wq", [D, CPB], bf16, kind="ExternalInput").ap()
    wk = nc.dram_tensor("wk", [D, CPB], bf16, kind="ExternalInput").ap()
    wv = nc.dram_tensor("wv", [D, CPB], bf16, kind="ExternalInput").ap()
    wo = nc.dram_tensor("wo", [CPB, D], bf16, kind="ExternalInput").ap()
    bqk = nc.dram_tensor("bqk", [128, 2 * CC], f32, kind="ExternalInput").ap()
    bv = nc.dram_tensor("bv", [1, CPB], bf16, kind="ExternalInput").ap()
    # (1-mask) per-partition scalars, one column per k-chunk
    m01p = nc.dram_tensor("m01p", [128, NK], f32, kind="ExternalInput").ap()
    out = nc.dram_tensor("out", [S, D], f32, kind="ExternalOutput").ap()

    with tile.TileContext(nc) as tc, ExitStack() as top:
        const = top.enter_context(tc.tile_pool(name="const", bufs=1))

        # ---- constant loads ----
        wq_sb = const.tile([128, DC, CPB], bf16, tag="wq")
        wk_sb = const.tile([128, DC, CPB], bf16, tag="wk")
        wv_sb = const.tile([128, DC, CPB], bf16, tag="wv")
        for w_sb, w_dram in ((wq_sb, wq), (wk_sb, wk), (wv_sb, wv)):
            nc.sync.dma_start(
                out=w_sb,
                in_=w_dram.rearrange("(dc p) c -> p dc c", p=128))
        # Wo per head-PAIR: [128, D] row-slices (contraction-128 Wo matmuls)
        wo_sb = const.tile([128, HL // 2, D], bf16, tag="wo")
        nc.sync.dma_start(out=wo_sb,
                          in_=wo.rearrange("(q p) e -> p q e", p=128))
        bqk_sb = const.tile([128, 2 * CC], f32, tag="bqk")
        nc.sync.dma_start(out=bqk_sb, in_=bqk)
        bv_sb = const.tile([1, CPB], bf16, tag="bv")
        nc.sync.dma_start(out=bv_sb, in_=bv)
        m01p_sb = const.tile([128, NK], f32, tag="m01p")
        nc.sync.dma_start(out=m01p_sb, in_=m01p)
        ones_sb = const.tile([1, 128], bf16, tag="ones")
        nc.gpsimd.memset(ones_sb, 1.0)

        qt_sb = [const.tile([128, S], bf16, tag=f"qt{c}", name=f"qt_sb{c}") for c in range(CC)]
        kt_sb = [const.tile([128, S], bf16, tag=f"kt{c}", name=f"kt_sb{c}") for c in range(CC)]
        v_sb = [const.tile([128, HL, HD + 1], bf16, tag=f"v{k}", name=f"v_sb{k}") for k in range(NK)]
        # ones/(1-mask) column of V', written once (constant across reps)
        for kc in range(NK):
            for h in range(HL):
                nc.gpsimd.tensor_copy(out=v_sb[kc][:, h, HD:HD + 1],
                                      in_=m01p_sb[:, kc:kc + 1])

        # xt tiles live in the never-closed const pool: reusing their SBUF
        # space would give later tile writers WAR/WAW waits on all 8 DMA
        # queues, exceeding HW sync-wait slots.
        xt_sb = [const.tile([128, S], bf16, tag=f"xt{dc}",
                            name=f"xt_sb{dc}") for dc in range(DC)]

        # PSUM budget (8 banks): lg 2x2 + cps 2 + ops/mm/vps shared 2 = 8
        lg_psum = top.enter_context(tc.tile_pool(name="lg", bufs=2, space="PSUM"))
        ctx_psum = top.enter_context(tc.tile_pool(name="cps", bufs=1, space="PSUM"))
        out_psum = top.enter_context(tc.tile_pool(name="ops", bufs=2, space="PSUM"))
        probs_pool = top.enter_context(tc.tile_pool(name="probs", bufs=8))
        probs1_pool = top.enter_context(tc.tile_pool(name="probs1", bufs=8))
        rec_pool = top.enter_context(tc.tile_pool(name="rec", bufs=6))
        ctx_pool = top.enter_context(tc.tile_pool(name="ctx", bufs=2))
        ctmp_pool = top.enter_context(tc.tile_pool(name="ctmp", bufs=2))
        outsb_pool = top.enter_context(tc.tile_pool(name="outsb", bufs=4))
        mm_psum = out_psum  # phase A accumulators share the ops slots

        loop = tc.For_i(0, reps, 1) if reps > 1 else nullcontext()
        with loop:
            # ---- phase A: input DMA + projections ----
            # per-(dc, sc) transfers, sc-major: the first projection tiles
            # (which need sc=0 of every dc) can start after ~6 small DMAs
            for sc in range(NQ4):
                for dc in range(DC):
                    nc.sync.dma_start(
                        out=xt_sb[dc][:, sc * 512:(sc + 1) * 512],
                        in_=xt[dc * 128:(dc + 1) * 128,
                               sc * 512:(sc + 1) * 512])

            # QT / KT psum-tile builder (one filler = one [128,512] tile)
            def qtkt_tile(iw, cc, sc):
                w_sb = wq_sb if iw == 0 else wk_sb
                qk = qt_sb if iw == 0 else kt_sb
                ps = mm_psum.tile([128, 512], f32, tag="ops",
                                  name=f"qkps_{iw}_{cc}_{sc}")
                for dc in range(DC):
                    nc.tensor.matmul(
                        ps,
                        lhsT=(w_sb[:, dc, cc * 128:(cc + 1) * 128]),
                        rhs=(xt_sb[dc][:, sc * 512:(sc + 1) * 512]),
                        start=(dc == 0), stop=(dc == DC - 1),
                    )
                nc.scalar.activation(
                    out=qk[cc][:, sc * 512:(sc + 1) * 512], in_=ps,
                    func=AF.Identity,
                    bias=bqk_sb[:, iw * CC + cc:iw * CC + cc + 1],
                )

            def build_qtkt_chunk(cc):
                for iw in range(2):
                    for sc in range(NQ4):
                        qtkt_tile(iw, cc, sc)

            build_qtkt_chunk(0)

            # V chunk: psum [128, 384]; extraction scales by (1-mask) per
            # k-partition (masked keys' V rows -> 0, matching the 0 in the
            # ones column, so they drop out of numerator and denominator).
            def v_chunk(kc):
                ps = mm_psum.tile([128, CPB], f32, tag="ops",
                                  name=f"vps_{kc}", padded_shape=[128, 512])
                for dc in range(DC):
                    nc.tensor.matmul(
                        ps,
                        lhsT=(xt_sb[dc][:, kc * 128:(kc + 1) * 128]),
                        rhs=(wv_sb[:, dc, :]),
                        start=(dc == 0), stop=False,
                    )
                nc.tensor.matmul(ps, lhsT=(ones_sb), rhs=(bv_sb),
                                 start=False, stop=True)
                nc.scalar.activation(
                    out=v_sb[kc][:, :, 0:HD],
                    in_=ps.rearrange("p (h d) -> p h d", h=HL),
                    func=AF.Copy, scale=m01p_sb[:, kc:kc + 1])

            for kc in range(NK):
                v_chunk(kc)

            # ---- phase B: attention with filler interleave ----
            fillers = []  # closures; one popped per kc slot

            def wo_group(ctx_pairs, wqc, qs):
                # two fillers per qs: e-halves (0,512) and (512,256)
                ob = outsb_pool.tile([128, D], f32, tag="ob",
                                     name=f"ob_{wqc}_{qs}")

                def half(e0, en, last):
                    def run():
                        ps = out_psum.tile([128, 512], f32, tag="ops",
                                           name=f"wops_{wqc}_{qs}_{e0}")
                        for p in range(HL // 2):
                            nc.tensor.matmul(
                                ps[:, 0:en],
                                lhsT=(ctx_pairs[p][:, qs * 128:(qs + 1) * 128]),
                                rhs=(wo_sb[:, p, e0:e0 + en]),
                                start=(p == 0), stop=(p == HL // 2 - 1),
                            )
                        nc.scalar.activation(out=ob[:, e0:e0 + en],
                                             in_=ps[:, 0:en], func=AF.Copy)
                        if last:
                            row = (wqc * 4 + qs) * 128
                            nc.sync.dma_start(out=out[row:row + 128, :], in_=ob)
                    return run
                return [half(0, 512, False), half(512, 256, True)]

            prev_ctx = prev_qc = None
            for qc in range(NQ4 if parts != "A" else 0):
                if prev_ctx is not None and parts != "noWo":
                    for qs in range(4):
                        fillers.extend(wo_group(prev_ctx, prev_qc, qs))
                # qt/kt chunk cc must be fully emitted before head-pair cc
                # reads it: pace chunk builders within the PRECEDING block.
                chunk_fill = {0: [], 1: [], 2: []}
                if qc == 0:
                    for cc in range(1, CC):
                        for iw in range(2):
                            for sc in range(NQ4):
                                chunk_fill[cc - 1].append(
                                    lambda iw=iw, cc=cc, sc=sc: qtkt_tile(iw, cc, sc))

                # spread Wo filler pops evenly over this qc's 48 kc slots
                slot, pops = [0], [0]
                nfill, nslots = [max(1, len(fillers))], [3 * NK]

                ctx_pairs = [ctx_pool.tile([128, 512], bf16, tag=f"ctxp{p}",
                                           name=f"ctxp{p}_{qc}")
                             for p in range(HL // 2)]
                for hp in range(HL // 2):
                    cfq = chunk_fill[hp]
                    ncf, cpops = len(cfq), 0
                    h0, h1 = 2 * hp, 2 * hp + 1
                    ccx = hp  # kt/qt chunk holding this head pair
                    cps = [ctx_psum.tile([HD + 1, 512], f32, tag=f"cps{i}",
                                         name=f"cps{i}_{qc}_{hp}")
                           for i in range(2)]
                    pend = []  # software-pipeline: PV trails logits by 3 kc
                    for kc in range(NK):
                        lgs = [lg_psum.tile([128, 512], f32, tag=f"lg{i}",
                                            name=f"lg{i}_{qc}_{hp}_{kc}")
                               for i in range(2)]
                        for i in range(2):
                            off = i * HD
                            nc.tensor.matmul(
                                lgs[i],
                                lhsT=(kt_sb[ccx][off:off + HD,
                                                  kc * 128:(kc + 1) * 128]),
                                rhs=(qt_sb[ccx][off:off + HD,
                                                 qc * 512:(qc + 1) * 512]),
                                start=True, stop=True,
                            )
                        pb0 = probs_pool.tile([128, 512], bf16, tag="pb0")
                        pb1 = probs1_pool.tile([128, 512], bf16, tag="pb1")
                        nc.scalar.activation(
                            out=pb0, in_=lgs[0], func=AF.Exp, scale=0.125,
                        )
                        nc.vector.tensor_scalar(
                            out=pb1.bitcast(i16), in0=lgs[1],
                            scalar1=SCH_SCALE, scalar2=SCH_BIAS,
                            op0=mybir.AluOpType.mult,
                            op1=mybir.AluOpType.add,
                        )
                        pend.append((kc, (pb0, pb1)))
                        # chunk builders: finish by slot 14 of this block
                        while cfq and cpops * 14 < (kc + 1) * ncf:
                            cfq.pop(0)()
                            cpops += 1
                        # Wo fillers: spread over the whole qc (48 slots)
                        slot[0] += 1
                        while fillers and pops[0] * nslots[0] < slot[0] * nfill[0]:
                            fillers.pop(0)()
                            pops[0] += 1
                        if len(pend) > 3:
                            k0, pbs = pend.pop(0)
                            _emit_pv(nc, cps, v_sb, pbs, h0, h1, k0, NK)
                    for k0, pbs in pend:
                        _emit_pv(nc, cps, v_sb, pbs, h0, h1, k0, NK)

                    for i, h in enumerate((h0, h1)):
                        rec = rec_pool.tile([1, 512], bf16, tag="rec")
                        with nc.allow_low_precision(reason="1/denom bf16: "
                                                    "0.4% scale noise ok"):
                            nc.vector.reciprocal(out=rec,
                                                 in_=cps[i][HD:HD + 1, :])
                        # broadcast 1/denom across 64 partitions on the idle
                        # Pool engine (SBUF->SBUF ucode broadcast)
                        rbc = rec_pool.tile([HD, 512], bf16, tag="rbc")
                        nc.gpsimd.partition_broadcast(rbc, rec, channels=HD)
                        if i == 0:
                            nc.vector.tensor_mul(ctx_pairs[hp][0:HD, :],
                                                 cps[i][0:HD, :], rbc)
                        else:
                            ctmp = ctmp_pool.tile([HD, 512], bf16, tag="ctmp")
                            nc.vector.tensor_mul(ctmp, cps[i][0:HD, :], rbc)
                            # partition-shift into rows 64..127 of the pair
                            nc.sync.dma_start(
                                out=ctx_pairs[hp][HD:2 * HD, :], in_=ctmp)
                prev_ctx, prev_qc = ctx_pairs, qc

            # drain remaining fillers, then the last q chunk's Wo (no
            # successor to hide in)
            for f in fillers:
                f()
            if prev_ctx is not None and parts != "noWo":
                for qs in range(4):
                    for f in wo_group(prev_ctx, prev_qc, qs):
                        f()

    nc.compile()
    return nc


def _emit_pv(nc, cps, v_sb, pbs, h0, h1, kc, nk):
    for i, h in enumerate((h0, h1)):
        nc.tensor.matmul(
            cps[i],
            lhsT=(v_sb[kc][:, h, :]),
            rhs=(pbs[i]),
            start=(kc == 0), stop=(kc == nk - 1),
        )


def _get_nc():
    if "nc" not in _cache:
        _cache["nc"] = _build_nc()
    return _cache["nc"]


def make_in_maps(x, mask, Wq, bq, Wk, bk, Wv, bv, Wo):
    """Per-core input maps for the SPMD kernel. Core i: batch i//2, heads i%2."""
    import ml_dtypes
    bf16 = ml_dtypes.bfloat16
    x = np.asarray(x, np.float32)
    mask = np.asarray(mask, np.float32)
    in_maps = []
    for core in range(8):
        b, g = divmod(core, 2)
        sl = slice(g * CPB, (g + 1) * CPB)
        bqk_arr = np.stack([np.asarray(bq, np.float32)[sl],
                            np.asarray(bk, np.float32)[sl]])  # [2, 384]
        m01 = (mask[b, 0, 0, :] == 0.0).astype(np.float32)  # 1 = keep
        in_maps.append({
            "xt": np.ascontiguousarray(x[b].T).astype(bf16),
            "wq": np.ascontiguousarray(np.asarray(Wq, np.float32)[:, sl]).astype(bf16),
            "wk": np.ascontiguousarray(np.asarray(Wk, np.float32)[:, sl]).astype(bf16),
            "wv": np.ascontiguousarray(np.asarray(Wv, np.float32)[:, sl]).astype(bf16),
            "wo": np.ascontiguousarray(np.asarray(Wo, np.float32)[sl, :]).astype(bf16),
            # [128, 2*CC]: per-partition bias columns, q then k
            "bqk": np.ascontiguousarray(
                bqk_arr.reshape(2, CC, 128).transpose(2, 0, 1).reshape(128, 2 * CC)),
            "bv": np.asarray(bv, np.float32)[sl].reshape(1, CPB).astype(bf16),
            "m01p": np.ascontiguousarray(m01.reshape(NK, 128).T),
        })
    return in_maps


def combine(results, bo):
    out = np.empty((4, S, D), np.float32)
    for b in range(4):
        out[b] = results[2 * b]["out"] + results[2 * b + 1]["out"] \
            + np.asarray(bo, np.float32)
    return out


def kernel(x, mask, Wq, bq, Wk, bk, Wv, bv, Wo, bo):
    from concourse.bass_utils import run_bass_kernel_spmd

    nc = _get_nc()
    in_maps = make_in_maps(x, mask, Wq, bq, Wk, bk, Wv, bv, Wo)
    res = run_bass_kernel_spmd(nc, in_maps, list(range(8))).results
    return combine(res, bo)


# revision 63
# speedup vs baseline: 1.1041x; 1.1041x over previous
"""Multi-head attention (B=4, S=2048, D=768, H=12) on 8 TRN2 NeuronCores.

Sharding: core i handles batch b = i//2 and head-group g = i%2 (6 heads of 64).
Each core computes Q/K/V projections for its head slice, attention, and a
partial output projection (row-slice of Wo). Host sums the two partials per
batch and adds bo.

Device layout (v3):
  - x fed pre-transposed as xT [D, S]; projections contract over D on the
    partition dim. Q, K produced transposed: QT/KT [384, S].
  - logitsT [k, q]: lhsT = KT_h [64, k-tile], rhs = QT_h [64, q-tile].
  - Mask folded into V instead of an exp bias: V rows and the softmax-ones
    column carry (1-mask), so masked keys drop out of both the PV numerator
    and the denominator. exp is then bias-free.
  - exp alternates WHOLE k-chunks between engines: even kc = exact Exp on
    ACT, odd kc = Schraudolph approximate exp on DVE (bf16 bits =
    trunc(23.083*logit + 16249.25) via an int16-bitcast tensor_scalar).
    Whole-chunk ownership halves cross-engine semaphore hops, which
    dominate the HW-vs-model gap (the attention loop is latency-bound).
  - V kept in natural [k, 6, 65] layout; the PV matmul accumulates
    ctxT [64, q] + denominator (row 64) per head in one PSUM group.
  - Normalization: DVE reciprocal of the denominator row, Pool (gpsimd)
    partition_broadcast across 64 partitions (no DMA round-trip), DVE
    multiply fused into the PSUM->SBUF extraction.
  - Output projection contracts over HEAD PAIRS (contraction 128): ctx of
    the odd head is partition-shifted into rows 64..127 of a pair tile via
    an SBUF->SBUF DMA, halving Wo matmul count vs per-head contraction 64.
  - Wo matmuls for q-chunk qc-1 and the qt/kt chunk builders are spread as
    fillers through the kc loops (ordering-safe pacing) so the PE never
    idles waiting on exp; PV trails logits by 5 kc (deep software pipeline
    to absorb real semaphore latency).
  - All matmul operands bf16 (fp32 PSUM accumulate).
"""

import numpy as np
from contextlib import ExitStack

S = 2048
D = 768
HL = 6  # heads per core
HD = 64
CPB = 384  # channels per core = HL * HD
DC = D // 128  # 6 contraction chunks
CC = CPB // 128  # 3 chunks of QT/KT partitions
NQ4 = S // 512  # 4 q chunks of 512
NK = S // 128  # 16 k chunks of 128
NEG_BIG = -1.0e9

# Schraudolph approx-exp constants (bf16 bit trick), softmax scale folded in.
SCH_SCALE = float(0.125 * 128 * np.log2(np.e))  # 23.0831...
SCH_BIAS = 16249.25

# exp split: ACT computes head 0 exactly; head 1 uses the Schraudolph
# approximation on Pool (even kc) / DVE (odd kc).

_cache = {}

import os as _os
_KBENCH_EXTRA = _os.environ.get("KBENCH_EXTRA", "")  # diagnostics only


def _build_nc(reps=1, parts="all"):
    import concourse.bass as bass
    import concourse.mybir as mybir
    import concourse.tile as tile
    from concourse import bacc
    from contextlib import nullcontext

    f32 = mybir.dt.float32
    bf16 = mybir.dt.bfloat16
    i16 = mybir.dt.int16
    AF = mybir.ActivationFunctionType

    nc = bacc.Bacc("TRN2", target_bir_lowering=False, debug=False,
                   enable_asserts=False)

    xt = nc.dram_tensor("xt", [D, S], bf16, kind="ExternalInput").ap()
    wq = nc.dram_tensor("wq", [D, CPB], bf16, kind="ExternalInput").ap()
    wk = nc.dram_tensor("wk", [D, CPB], bf16, kind="ExternalInput").ap()
    wv = nc.dram_tensor("wv", [D, CPB], bf16, kind="ExternalInput").ap()
    wo = nc.dram_tensor("wo", [CPB, D], bf16, kind="ExternalInput").ap()
    bqk = nc.dram_tensor("bqk", [128, 2 * CC], f32, kind="ExternalInput").ap()
    bv = nc.dram_tensor("bv", [1, CPB], bf16, kind="ExternalInput").ap()
    # (1-mask) per-partition scalars, one column per k-chunk
    m01p = nc.dram_tensor("m01p", [128, NK], f32, kind="ExternalInput").ap()
    out = nc.dram_tensor("out", [S, D], f32, kind="ExternalOutput").ap()

    with tile.TileContext(nc) as tc, ExitStack() as top:
        const = top.enter_context(tc.tile_pool(name="const", bufs=1))

        # ---- constant loads ----
        # wq first: the first projection tiles need only wq + xt[sc=0]
        wq_sb = const.tile([128, DC, CPB], bf16, tag="wq")
        wk_sb = const.tile([128, DC, CPB], bf16, tag="wk")
        wv_sb = const.tile([128, DC, CPB], bf16, tag="wv")
        nc.sync.dma_start(out=wq_sb,
                          in_=wq.rearrange("(dc p) c -> p dc c", p=128))
        wo_sb = const.tile([128, HL // 2, D], bf16, tag="wo")
        bqk_sb = const.tile([128, 2 * CC], f32, tag="bqk")
        bv_sb = const.tile([1, CPB], bf16, tag="bv")
        m01p_sb = const.tile([128, NK], f32, tag="m01p")
        ones_sb = const.tile([1, 128], bf16, tag="ones")
        nc.gpsimd.memset(ones_sb, 1.0)

        def load_consts_rest():
            for w_sb, w_dram in ((wk_sb, wk), (wv_sb, wv)):
                nc.sync.dma_start(
                    out=w_sb,
                    in_=w_dram.rearrange("(dc p) c -> p dc c", p=128))
            # Wo per head-PAIR [128, D] rows (contraction-128 Wo matmuls)
            nc.sync.dma_start(out=wo_sb,
                              in_=wo.rearrange("(q p) e -> p q e", p=128))
            nc.sync.dma_start(out=bqk_sb, in_=bqk)
            nc.sync.dma_start(out=bv_sb, in_=bv)
            nc.sync.dma_start(out=m01p_sb, in_=m01p)

        load_consts_rest()
        qt_sb = [const.tile([128, S], bf16, tag=f"qt{c}", name=f"qt_sb{c}") for c in range(CC)]
        kt_sb = [const.tile([128, S], bf16, tag=f"kt{c}", name=f"kt_sb{c}") for c in range(CC)]
        v_sb = [const.tile([128, HL, HD + 1], bf16, tag=f"v{k}", name=f"v_sb{k}") for k in range(NK)]
        # ones/(1-mask) column of V', written once (constant across reps)
        for kc in range(NK):
            for h in range(HL):
                nc.gpsimd.tensor_copy(out=v_sb[kc][:, h, HD:HD + 1],
                                      in_=m01p_sb[:, kc:kc + 1])

        # x lives in one [128, dc, S] tile so a single DMA per s-chunk can
        # carry all 6 contraction chunks (4 dispatches instead of 24).
        xtall = const.tile([128, DC, S], bf16, tag="xtall")
        xt_sb = [xtall[:, dc, :] for dc in range(DC)]

        # PSUM budget (8 banks): lg 2x2 + cps 2 + ops/mm/vps shared 2 = 8
        lg0b = int(_os.environ.get("LG0B", "2"))
        opsb = int(_os.environ.get("OPSB", "2"))
        lg_psum = top.enter_context(tc.tile_pool(name="lg", bufs=lg0b, space="PSUM"))
        ctx_psum = top.enter_context(tc.tile_pool(name="cps", bufs=1, space="PSUM"))
        out_psum = top.enter_context(tc.tile_pool(name="ops", bufs=opsb, space="PSUM"))
        probs_pool = top.enter_context(tc.tile_pool(name="probs", bufs=8))
        probs1_pool = top.enter_context(tc.tile_pool(name="probs1", bufs=8))
        rec_pool = top.enter_context(tc.tile_pool(name="rec", bufs=6))
        ctx_pool = top.enter_context(tc.tile_pool(name="ctx", bufs=2))
        ctmp_pool = top.enter_context(tc.tile_pool(name="ctmp", bufs=2))
        outsb_pool = top.enter_context(tc.tile_pool(name="outsb", bufs=4))
        mm_psum = out_psum  # phase A accumulators share the ops slots

        loop = tc.For_i(0, reps, 1) if reps > 1 else nullcontext()
        with loop:
            # ---- phase A: input DMA + projections ----
            # one DMA per s-chunk covering all dc (first projection tiles
            # need only the sc=0 transfer)
            if _os.environ.get("XTDMA", "big") == "big":
                for sc in range(NQ4):
                    nc.sync.dma_start(
                        out=xtall[:, :, sc * 512:(sc + 1) * 512],
                        in_=xt.rearrange("(dc p) s -> p dc s", p=128)[
                            :, :, sc * 512:(sc + 1) * 512])
            else:
                for sc in range(NQ4):
                    for dc in range(DC):
                        nc.sync.dma_start(
                            out=xtall[:, dc, sc * 512:(sc + 1) * 512],
                            in_=xt[dc * 128:(dc + 1) * 128,
                                   sc * 512:(sc + 1) * 512])

            # QT / KT psum-tile builder (one filler = one [128,512] tile)
            def qtkt_tile(iw, cc, sc):
                w_sb = wq_sb if iw == 0 else wk_sb
                qk = qt_sb if iw == 0 else kt_sb
                ps = mm_psum.tile([128, 512], f32, tag="ops",
                                  name=f"qkps_{iw}_{cc}_{sc}")
                for dc in range(DC):
                    nc.tensor.matmul(
                        ps,
                        lhsT=(w_sb[:, dc, cc * 128:(cc + 1) * 128]),
                        rhs=(xt_sb[dc][:, sc * 512:(sc + 1) * 512]),
                        start=(dc == 0), stop=(dc == DC - 1),
                    )
                nc.scalar.activation(
                    out=qk[cc][:, sc * 512:(sc + 1) * 512], in_=ps,
                    func=AF.Identity,
                    bias=bqk_sb[:, iw * CC + cc:iw * CC + cc + 1],
                )

            def build_qtkt_chunk(cc):
                for iw in range(2):
                    for sc in range(NQ4):
                        qtkt_tile(iw, cc, sc)

            build_qtkt_chunk(0)

            # V chunk: psum [128, 384]; extraction scales by (1-mask) per
            # k-partition (masked keys' V rows -> 0, matching the 0 in the
            # ones column, so they drop out of numerator and denominator).
            def v_chunk(kc):
                ps = mm_psum.tile([128, CPB], f32, tag="ops",
                                  name=f"vps_{kc}", padded_shape=[128, 512])
                for dc in range(DC):
                    nc.tensor.matmul(
                        ps,
                        lhsT=(xt_sb[dc][:, kc * 128:(kc + 1) * 128]),
                        rhs=(wv_sb[:, dc, :]),
                        start=(dc == 0), stop=False,
                    )
                nc.tensor.matmul(ps, lhsT=(ones_sb), rhs=(bv_sb),
                                 start=False, stop=True)
                nc.scalar.activation(
                    out=v_sb[kc][:, :, 0:HD],
                    in_=ps.rearrange("p (h d) -> p h d", h=HL),
                    func=AF.Copy, scale=m01p_sb[:, kc:kc + 1])

            # V chunks become qc0-hp0 fillers: V[kc] is emitted at slot kc,
            # consumed by PV[kc] at slot kc+3 (pend depth 3).
            vfill = _os.environ.get("VFILL", "0") == "1"
            if parts == "A" or not vfill:
                for kc in range(NK):
                    v_chunk(kc)

            # ---- phase B: attention with filler interleave ----
            fillers = []  # closures; one popped per kc slot

            def wo_group(ctx_pairs, wqc, qs):
                # two fillers per qs: e-halves (0,512) and (512,256)
                ob = outsb_pool.tile([128, D], f32, tag="ob",
                                     name=f"ob_{wqc}_{qs}")

                def half(e0, en, last):
                    def run():
                        ps = out_psum.tile([128, 512], f32, tag="ops",
                                           name=f"wops_{wqc}_{qs}_{e0}")
                        for p in range(HL // 2):
                            nc.tensor.matmul(
                                ps[:, 0:en],
                                lhsT=(ctx_pairs[p][:, qs * 128:(qs + 1) * 128]),
                                rhs=(wo_sb[:, p, e0:e0 + en]),
                                start=(p == 0), stop=(p == HL // 2 - 1),
                            )
                        nc.scalar.activation(out=ob[:, e0:e0 + en],
                                             in_=ps[:, 0:en], func=AF.Copy)
                        if last:
                            row = (wqc * 4 + qs) * 128
                            nc.sync.dma_start(out=out[row:row + 128, :], in_=ob)
                    return run
                return [half(0, 512, False), half(512, 256, True)]

            prev_ctx = prev_qc = None
            for qc in range(NQ4 if parts != "A" else 0):
                if prev_ctx is not None and parts != "noWo":
                    for qs in range(4):
                        fillers.extend(wo_group(prev_ctx, prev_qc, qs))
                # qt/kt chunk cc must be fully emitted before head-pair cc
                # reads it: pace chunk builders within the PRECEDING block.
                # qc0-hp0 carries the V chunks (1 per slot, just ahead of
                # their PV consumers); cc1 -> hp1, cc2 -> hp2.
                chunk_fill = {0: [], 1: [], 2: []}
                if qc == 0:
                    if vfill:
                        cc1 = [(iw, sc) for iw in range(2) for sc in range(NQ4)]
                        for kc in range(NK):
                            chunk_fill[0].append(lambda kc=kc: v_chunk(kc))
                            if kc % 2 == 1 and cc1:
                                iw, sc = cc1.pop(0)
                                chunk_fill[0].append(
                                    lambda iw=iw, sc=sc: qtkt_tile(iw, 1, sc))
                        for iw in range(2):
                            for sc in range(NQ4):
                                chunk_fill[1].append(
                                    lambda iw=iw, sc=sc: qtkt_tile(iw, 2, sc))
                    else:
                        for cc in range(1, CC):
                            for iw in range(2):
                                for sc in range(NQ4):
                                    chunk_fill[cc - 1].append(
                                        lambda iw=iw, cc=cc, sc=sc:
                                        qtkt_tile(iw, cc, sc))

                # spread Wo filler pops evenly over this qc's 48 kc slots
                slot, pops = [0], [0]
                nfill, nslots = [max(1, len(fillers))], [3 * NK]

                ctx_pairs = [ctx_pool.tile([128, 512], bf16, tag=f"ctxp{p}",
                                           name=f"ctxp{p}_{qc}")
                             for p in range(HL // 2)]
                for hp in range(HL // 2):
                    cfq = chunk_fill[hp]
                    ncf, cpops = len(cfq), 0
                    h0, h1 = 2 * hp, 2 * hp + 1
                    ccx = hp  # kt/qt chunk holding this head pair
                    cps = [ctx_psum.tile([HD + 1, 512], f32, tag=f"cps{i}",
                                         name=f"cps{i}_{qc}_{hp}")
                           for i in range(2)]
                    pend = []  # software-pipeline: PV trails logits
                    for kc in range(NK):
                        # one engine owns the whole kc (both heads): even kc
                        # ACT exact exp, odd kc DVE Schraudolph. Halves the
                        # cross-engine sync hops vs a per-head split.
                        lg = lg_psum.tile([128, 2, 512], f32, tag="lg",
                                          name=f"lg_{qc}_{hp}_{kc}")
                        for i in range(2):
                            off = i * HD
                            nc.tensor.matmul(
                                lg[:, i, :],
                                lhsT=(kt_sb[ccx][off:off + HD,
                                                  kc * 128:(kc + 1) * 128]),
                                rhs=(qt_sb[ccx][off:off + HD,
                                                 qc * 512:(qc + 1) * 512]),
                                start=True, stop=True,
                            )
                        pb = probs_pool.tile([128, 2, 512], bf16, tag="pb")
                        egrp = int(_os.environ.get("EGRP", "1"))
                        if (kc // egrp) % 2 == 0:
                            nc.scalar.activation(
                                out=pb, in_=lg, func=AF.Exp, scale=0.125,
                            )
                        else:
                            nc.vector.tensor_scalar(
                                out=pb.bitcast(i16), in0=lg,
                                scalar1=SCH_SCALE, scalar2=SCH_BIAS,
                                op0=mybir.AluOpType.mult,
                                op1=mybir.AluOpType.add,
                            )
                        pend.append((kc, (pb[:, 0, :], pb[:, 1, :])))
                        # chunk builders: finish by slot 14 of this block
                        while cfq and cpops * 14 < (kc + 1) * ncf:
                            cfq.pop(0)()
                            cpops += 1
                        # Wo fillers: spread over the whole qc (48 slots)
                        slot[0] += 1
                        while fillers and pops[0] * nslots[0] < slot[0] * nfill[0]:
                            fillers.pop(0)()
                            pops[0] += 1
                        if len(pend) > int(_os.environ.get("PENDD", "5")):
                            k0, pbs = pend.pop(0)
                            _emit_pv(nc, cps, v_sb, pbs, h0, h1, k0, NK)
                    for k0, pbs in pend:
                        _emit_pv(nc, cps, v_sb, pbs, h0, h1, k0, NK)

                    for i, h in enumerate((h0, h1)):
                        rec = rec_pool.tile([1, 512], bf16, tag="rec")
                        with nc.allow_low_precision(reason="1/denom bf16: "
                                                    "0.4% scale noise ok"):
                            nc.vector.reciprocal(out=rec,
                                                 in_=cps[i][HD:HD + 1, :])
                        # broadcast 1/denom across 64 partitions on the idle
                        # Pool engine (SBUF->SBUF ucode broadcast)
                        rbc = rec_pool.tile([HD, 512], bf16, tag="rbc")
                        nc.gpsimd.partition_broadcast(rbc, rec, channels=HD)
                        if i == 0:
                            nc.vector.tensor_mul(ctx_pairs[hp][0:HD, :],
                                                 cps[i][0:HD, :], rbc)
                        else:
                            ctmp = ctmp_pool.tile([HD, 512], bf16, tag="ctmp")
                            nc.vector.tensor_mul(ctmp, cps[i][0:HD, :], rbc)
                            # partition-shift into rows 64..127 of the pair
                            nc.sync.dma_start(
                                out=ctx_pairs[hp][HD:2 * HD, :], in_=ctmp)
                prev_ctx, prev_qc = ctx_pairs, qc

            # drain remaining fillers, then the last q chunk's Wo (no
            # successor to hide in)
            for f in fillers:
                f()
            if prev_ctx is not None and parts != "noWo":
                for qs in range(4):
                    for f in wo_group(prev_ctx, prev_qc, qs):
                        f()

    nc.compile()
    return nc


def _emit_pv(nc, cps, v_sb, pbs, h0, h1, kc, nk):
    for i, h in enumerate((h0, h1)):
        nc.tensor.matmul(
            cps[i],
            lhsT=(v_sb[kc][:, h, :]),
            rhs=(pbs[i]),
            start=(kc == 0), stop=(kc == nk - 1),
        )


def _get_nc():
    if "nc" not in _cache:
        _cache["nc"] = _build_nc()
    return _cache["nc"]


def make_in_maps(x, mask, Wq, bq, Wk, bk, Wv, bv, Wo):
    """Per-core input maps for the SPMD kernel. Core i: batch i//2, heads i%2."""
    import ml_dtypes
    bf16 = ml_dtypes.bfloat16
    x = np.asarray(x, np.float32)
    mask = np.asarray(mask, np.float32)
    in_maps = []
    for core in range(8):
        b, g = divmod(core, 2)
        sl = slice(g * CPB, (g + 1) * CPB)
        bqk_arr = np.stack([np.asarray(bq, np.float32)[sl],
                            np.asarray(bk, np.float32)[sl]])  # [2, 384]
        m01 = (mask[b, 0, 0, :] == 0.0).astype(np.float32)  # 1 = keep
        in_maps.append({
            "xt": np.ascontiguousarray(x[b].T).astype(bf16),
            "wq": np.ascontiguousarray(np.asarray(Wq, np.float32)[:, sl]).astype(bf16),
            "wk": np.ascontiguousarray(np.asarray(Wk, np.float32)[:, sl]).astype(bf16),
            "wv": np.ascontiguousarray(np.asarray(Wv, np.float32)[:, sl]).astype(bf16),
            "wo": np.ascontiguousarray(np.asarray(Wo, np.float32)[sl, :]).astype(bf16),
            # [128, 2*CC]: per-partition bias columns, q then k
            "bqk": np.ascontiguousarray(
                bqk_arr.reshape(2, CC, 128).transpose(2, 0, 1).reshape(128, 2 * CC)),
            "bv": np.asarray(bv, np.float32)[sl].reshape(1, CPB).astype(bf16),
            "m01p": np.ascontiguousarray(m01.reshape(NK, 128).T),
        })
    return in_maps


def combine(results, bo):
    out = np.empty((4, S, D), np.float32)
    for b in range(4):
        out[b] = results[2 * b]["out"] + results[2 * b + 1]["out"] \
            + np.asarray(bo, np.float32)
    return out


def kernel(x, mask, Wq, bq, Wk, bk, Wv, bv, Wo, bo):
    from concourse.bass_utils import run_bass_kernel_spmd

    nc = _get_nc()
    in_maps = make_in_maps(x, mask, Wq, bq, Wk, bk, Wv, bv, Wo)
    res = run_bass_kernel_spmd(nc, in_maps, list(range(8))).results
    return combine(res, bo)
